# revision 6
# baseline (speedup 1.0000x reference)
"""CGConv GNN layer (CGCNNet + L1 sum head) on 8 Trainium2 NeuronCores.

Strategy (v3 — streaming, gather-free, software-pipelined):
  - Host sorts edges by destination node; each core owns a contiguous range of
    destination nodes (49 windows of 128 nodes), so segment-sums complete
    locally and no collectives are needed.
  - Host streams pre-transposed per-edge tiles: zA = [x_dst | x_src]^T
    (128 rows) and zB = [edge_attr | ones]^T (33 rows), sorted into
    (core, window, slot) order. No on-device gathers at all.
  - Per 128-edge tile: pre = zA^T @ W_A + zB^T @ W_B accumulated in PSUM
    (CGConv weights restacked so gate|core come out side by side, bias folded
    in via the ones row).
  - Nonlinearity with one resident ACT table (exp_and_others: tanh + exp):
      gate2 = 1 + tanh(pre_g / 2)            (= 2*sigmoid(pre_g))
      v     = exp(pre_c); sp = ln(1 + v)     (ln batched once per window)
      msg2  = gate2 * sp                     (one fused DVE op; = 2*msg)
  - Segment-sum via one-hot scatter matmuls (S built by DVE is_equal against
    an iota tile); the 2x is folded out in the epilogue:
      h = relu(0.5*agg2 + x_own), hacc += h.
  - Window stages are issued with a 1-window skew (stage A of window w+1
    ahead of the scatter of window w) so the tensor engine never waits on the
    ACT ln / DVE msg chain.
  - Final: pooled = ones^T @ hacc on PE; host sums the 8 per-core [64]
    vectors and applies the dense head.
"""

import os
import sys
import numpy as np

sys.path.insert(0, "/opt/trn_rl_repo")

import ml_dtypes

P = 128
NB = 4                  # tiles per PSUM batch

LAST_RESULTS = None     # test harness reads exec_time_ns from here


def _patch_tile_drain():
    """This walrus build rejects >1 semaphore wait on the tail-drain TPB_CTRL
    instruction. Split the waits across preceding NOPs."""
    import concourse.tile as tile_mod
    from concourse import mybir
    from concourse.vector_clock import ScopedClock

    if getattr(tile_mod.TileContext, "_drain_patched", False):
        return

    def _drain_and_barrier(self, tick_clock, wait_clock):
        nc = self.nc
        drain_inst = nc.sync.drain()
        wait_clock.add_sem_waits(
            drain_inst.ins, ScopedClock({None: tick_clock.global_clock})
        )
        si = drain_inst.ins.sync_info
        waits = list(si.on_wait or [])
        if len(waits) > 1:
            si.on_wait = waits[:1]
            extra = waits[1:]
            bb = nc.cur_bb.bb
            insts = bb.instructions
            carriers = []
            for w in extra:
                ni = nc.sync.nop(nofuse=True, hint="drain_wait_split")
                ni.ins.sync_info = mybir.SyncInfo(on_wait=[w], on_update=[])
                carriers.append(ni.ins)
            di = insts.index(drain_inst.ins)
            for c in carriers:
                insts.remove(c)
            insts[di:di] = carriers

        nc.all_engine_barrier()
        assert self.sems is not None
        popped = nc._tile_sem_poison_stack.pop()
        assert popped is self._sem_poison
        nc.clear_and_free_semaphores(list(self.sems.allocated().values()))
        nc.all_engine_barrier()

    tile_mod.TileContext._drain_and_barrier = _drain_and_barrier
    tile_mod.TileContext._drain_patched = True


def _chunks(n, size):
    out = []
    t = 0
    while t < n:
        out.append((t, min(t + size, n)))
        t += size
    return out


def _host_prep(inputs, ncores):
    bf16 = ml_dtypes.bfloat16
    x = np.asarray(inputs["x"], dtype=np.float32)
    ei = np.asarray(inputs["edge_index"], dtype=np.int64)
    ea = np.asarray(inputs["edge_attr"], dtype=np.float32)
    W_f = np.asarray(inputs["W_f"], dtype=np.float32)
    b_f = np.asarray(inputs["b_f"], dtype=np.float32)
    W_s = np.asarray(inputs["W_s"], dtype=np.float32)
    b_s = np.asarray(inputs["b_s"], dtype=np.float32)

    N, F = x.shape
    E = ei.shape[1]
    D = ea.shape[1]
    GC = 2 * F  # gate|core width

    nodes_pc = -(-((N + ncores - 1) // ncores) // P) * P   # per-core range
    wpc = nodes_pc // P
    nwin = ncores * wpc

    src, dst = ei[0], ei[1]
    core = dst // nodes_pc
    w_of = (dst % nodes_pc) // P
    d_rel = dst % P

    key = core * wpc + w_of
    order = np.argsort(key, kind="stable")
    key_s = key[order]
    src_s, dst_s, drel_s = src[order], dst[order], d_rel[order]

    counts = np.bincount(key_s, minlength=nwin).reshape(ncores, wpc)
    # per-window tile count: max over cores (SPMD: one program for all cores)
    nt_w = np.maximum(1, -(-counts.max(axis=0) // P))       # [wpc]
    off_w = np.zeros(wpc + 1, dtype=np.int64)
    np.cumsum(nt_w * P, out=off_w[1:])
    tot = int(off_w[-1])                                    # slots per core

    # slot index: window offset + position within window
    starts = np.zeros(nwin + 1, dtype=np.int64)
    np.cumsum(counts.reshape(-1), out=starts[1:])
    within = np.arange(E, dtype=np.int64) - starts[key_s]
    core_s = key_s // wpc
    w_s = key_s % wpc
    fidx = core_s * tot + off_w[w_s] + within

    x_bf = x.astype(bf16)
    ZAf = np.zeros((ncores * tot, P), dtype=bf16)
    ZAf[fidx, 0:F] = x_bf[dst_s]
    ZAf[fidx, F:2 * F] = x_bf[src_s]
    ZBf = np.zeros((ncores * tot, D + 1), dtype=bf16)
    ZBf[fidx, 0:D] = ea[order].astype(bf16)
    ZBf[:, D] = 1.0
    drelf = np.full(ncores * tot, -1.0, dtype=bf16)
    drelf[fidx] = drel_s.astype(bf16)

    # weights: rows = z dims, cols = [gate | core]
    W_A = np.concatenate([W_f[0:2 * F], W_s[0:2 * F]], axis=1).astype(bf16)
    W_B = np.concatenate(
        [np.concatenate([W_f[2 * F:], W_s[2 * F:]], axis=1),
         np.concatenate([b_f, b_s])[None, :]], axis=0).astype(bf16)

    iota_rep = np.tile(np.arange(P, dtype=np.float32), NB) \
        .astype(bf16)[None, :].repeat(P, axis=0)               # [P, NB*P]

    in_maps = []
    for c in range(ncores):
        lo, hi = c * tot, (c + 1) * tot
        zA = np.ascontiguousarray(ZAf[lo:hi].T)                # [128, tot]
        zB = np.ascontiguousarray(ZBf[lo:hi].T)                # [33, tot]
        drw = np.ascontiguousarray(
            drelf[lo:hi].reshape(tot // P, P).T)               # [128, tot//P]
        nlo = c * nodes_pc
        nhi = min(N, nlo + nodes_pc)
        xo = np.zeros((nodes_pc, F), dtype=bf16)
        if nhi > nlo:
            xo[: nhi - nlo] = x_bf[nlo:nhi]
        in_maps.append({
            "zA": zA,
            "zB": zB,
            "drw": drw,
            "x_own": np.ascontiguousarray(xo.reshape(wpc, P, F)),
            "W_A": W_A,
            "W_B": W_B,
            "iota": iota_rep,
        })

    cfg = dict(N=N, E=E, F=F, D=D, GC=GC, ncores=ncores, nodes_pc=nodes_pc,
               wpc=wpc, nt_w=[int(v) for v in nt_w],
               off_w=[int(v) for v in off_w], tot=tot)
    return in_maps, cfg


def _build_program(cfg):
    import concourse.bass as bass
    import concourse.tile as tile
    from concourse import bacc, mybir
    from contextlib import ExitStack

    _patch_tile_drain()

    F, GC, D = cfg["F"], cfg["GC"], cfg["D"]
    wpc, tot = cfg["wpc"], cfg["tot"]
    nt_w, off_w = cfg["nt_w"], cfg["off_w"]
    ncores = cfg["ncores"]
    ZB_R = D + 1
    f32, bf16 = mybir.dt.float32, mybir.dt.bfloat16
    AF = mybir.ActivationFunctionType
    ALU = mybir.AluOpType
    nt_max = max(nt_w)

    nc = bacc.Bacc("TRN2", target_bir_lowering=False, debug=False,
                   num_devices=ncores)

    t_in = {}
    t_in["zA"] = nc.dram_tensor("zA", [P, tot], bf16, kind="ExternalInput")
    t_in["zB"] = nc.dram_tensor("zB", [ZB_R, tot], bf16, kind="ExternalInput")
    t_in["drw"] = nc.dram_tensor("drw", [P, tot // P], bf16, kind="ExternalInput")
    t_in["x_own"] = nc.dram_tensor("x_own", [wpc, P, F], bf16, kind="ExternalInput")
    t_in["W_A"] = nc.dram_tensor("W_A", [P, GC], bf16, kind="ExternalInput")
    t_in["W_B"] = nc.dram_tensor("W_B", [ZB_R, GC], bf16, kind="ExternalInput")
    t_in["iota"] = nc.dram_tensor("iota", [P, NB * P], bf16, kind="ExternalInput")

    out_pooled = nc.dram_tensor("out_pooled", [1, F], f32, kind="ExternalOutput")

    with tile.TileContext(nc) as tc:
        with ExitStack() as ctx:
            cpool = ctx.enter_context(tc.tile_pool(name="consts", bufs=1))
            w_a_sb = cpool.tile([P, GC], bf16)
            nc.scalar.dma_start(w_a_sb[:], t_in["W_A"].ap()[:, :])
            w_b_sb = cpool.tile([ZB_R, GC], bf16)
            nc.scalar.dma_start(w_b_sb[:], t_in["W_B"].ap()[:, :])
            iota_sb = cpool.tile([P, NB, P], bf16)
            nc.scalar.dma_start(iota_sb[:], t_in["iota"].ap()[:, :])
            ones_sb = cpool.tile([P, 1], f32)
            nc.vector.memset(ones_sb[:], 1.0)
            hacc = cpool.tile([P, F], f32)
            nc.vector.memset(hacc[:], 0.0)

            zA_pool = ctx.enter_context(tc.tile_pool(name="zA", bufs=3))
            zB_pool = ctx.enter_context(tc.tile_pool(name="zB", bufs=3))
            dpool = ctx.enter_context(tc.tile_pool(name="drw", bufs=3))
            xpool = ctx.enter_context(tc.tile_pool(name="xw", bufs=3))
            tpool = ctx.enter_context(tc.tile_pool(name="tslab", bufs=3))
            vpool = ctx.enter_context(tc.tile_pool(name="vslab", bufs=3))
            sppool = ctx.enter_context(tc.tile_pool(name="spslab", bufs=3))
            mpool = ctx.enter_context(tc.tile_pool(name="msgslab", bufs=3))
            stpool = ctx.enter_context(tc.tile_pool(name="st", bufs=3))
            epool = ctx.enter_context(tc.tile_pool(name="epi", bufs=3))
            pre_ps = ctx.enter_context(
                tc.tile_pool(name="preps", bufs=2, space="PSUM"))
            agg_ps = ctx.enter_context(
                tc.tile_pool(name="aggps", bufs=3, space="PSUM"))
            pool_ps = ctx.enter_context(
                tc.tile_pool(name="poolps", bufs=1, space="PSUM"))

            state = {}  # per in-flight window: tiles for stage C

            def stage_a(w):
                nt = nt_w[w]
                o0 = off_w[w]
                zA_t = zA_pool.tile([P, nt * P], bf16, tag="zA")
                nc.sync.dma_start(zA_t[:], t_in["zA"].ap()[:, o0:o0 + nt * P])
                zB_t = zB_pool.tile([ZB_R, nt * P], bf16, tag="zB")
                nc.sync.dma_start(zB_t[:], t_in["zB"].ap()[:, o0:o0 + nt * P])
                drw = dpool.tile([P, nt, 1], bf16, tag="drw")
                nc.sync.dma_start(
                    drw[:], t_in["drw"].ap()[:, o0 // P:o0 // P + nt])
                xw = xpool.tile([P, F], bf16, tag="xw")
                nc.sync.dma_start(xw[:], t_in["x_own"].ap()[w])

                tsl = tpool.tile([P, nt, F], bf16, tag="t")
                vsl = vpool.tile([P, nt, F], bf16, tag="v")
                for t0, t1 in _chunks(nt, NB):
                    nb = t1 - t0
                    pp = pre_ps.tile([P, NB, GC], f32, tag="pre")
                    for t in range(t0, t1):
                        i = t - t0
                        nc.tensor.matmul(pp[:, i, :],
                                         lhsT=zA_t[:, t * P:(t + 1) * P],
                                         rhs=w_a_sb[:], start=True, stop=False)
                        nc.tensor.matmul(pp[:, i, :],
                                         lhsT=zB_t[:, t * P:(t + 1) * P],
                                         rhs=w_b_sb[:], start=False, stop=True)
                    nc.scalar.activation(tsl[:, t0:t1, :], pp[:, :nb, 0:F],
                                         AF.Tanh, scale=0.5)
                    nc.scalar.activation(vsl[:, t0:t1, :], pp[:, :nb, F:GC],
                                         AF.Exp)
                state[w] = dict(drw=drw, xw=xw, tsl=tsl, vsl=vsl)

            def stage_b(w):
                s = state[w]
                nt = nt_w[w]
                spl = sppool.tile([P, nt, F], bf16, tag="sp")
                nc.scalar.activation(spl[:], s["vsl"][:], AF.Ln, bias=1.0)
                s["spl"] = spl

            def stage_c(w):
                s = state.pop(w)
                nt = nt_w[w]
                # one-hot tiles first (no cross-engine deps) so DVE never
                # blocks the scatter behind the next window's msg wait
                sts = []
                for t0, t1 in _chunks(nt, NB):
                    nb = t1 - t0
                    st = stpool.tile([P, NB, P], bf16, tag="st")
                    nc.vector.tensor_tensor(
                        st[:, :nb, :], iota_sb[:, :nb, :],
                        s["drw"][:, t0:t1, :].to_broadcast([P, nb, P]),
                        op=ALU.is_equal)
                    sts.append(st)
                msl = mpool.tile([P, nt, F], bf16, tag="msg")
                nc.vector.scalar_tensor_tensor(
                    msl[:], s["tsl"][:], 1.0, s["spl"][:],
                    op0=ALU.add, op1=ALU.mult)

                agg = agg_ps.tile([P, F], f32, tag="agg")
                for bi, (t0, t1) in enumerate(_chunks(nt, NB)):
                    st = sts[bi]
                    for t in range(t0, t1):
                        nc.tensor.matmul(agg[:], lhsT=st[:, t - t0, :],
                                         rhs=msl[:, t, :],
                                         start=(t == 0), stop=(t == nt - 1))

                hsum = epool.tile([P, F], f32, tag="hsum")
                nc.vector.scalar_tensor_tensor(
                    hsum[:], agg[:], 0.5, s["xw"][:], op0=ALU.mult, op1=ALU.add)
                h = epool.tile([P, F], f32, tag="h")
                nc.vector.tensor_scalar_max(h[:], hsum[:], 0.0)
                nc.vector.tensor_tensor(hacc[:], hacc[:], h[:], op=ALU.add)

            for w in range(wpc + 1):
                if w < wpc:
                    stage_a(w)
                    stage_b(w)
                if w >= 1:
                    stage_c(w - 1)

            # pool over partitions
            pooled_ps = pool_ps.tile([1, F], f32, tag="pool")
            nc.tensor.matmul(pooled_ps[:], lhsT=ones_sb[:], rhs=hacc[:],
                             start=True, stop=True)
            pooled_sb = epool.tile([1, F], f32, tag="pooled")
            nc.vector.tensor_copy(pooled_sb[:], pooled_ps[:])
            nc.scalar.dma_start(out_pooled.ap()[:, :], pooled_sb[:])

    nc.compile()
    return nc


def kernel(**inputs):
    global LAST_RESULTS
    from concourse.bass_utils import run_bass_kernel_spmd

    ncores = 8
    in_maps, cfg = _host_prep(inputs, ncores)
    nc = _build_program(cfg)
    trace = bool(os.environ.get("BASS_TRACE"))
    res = run_bass_kernel_spmd(nc, in_maps, list(range(ncores)), trace=trace)
    LAST_RESULTS = res

    pooled = np.zeros(cfg["F"], dtype=np.float64)
    for c in range(ncores):
        pooled += res.results[c]["out_pooled"][0].astype(np.float64)
    W_dense = np.asarray(inputs["W_dense"], dtype=np.float64)
    b_dense = np.asarray(inputs["b_dense"], dtype=np.float64)
    out = pooled @ W_dense + b_dense
    return out.astype(np.float32)


# revision 9
# speedup vs baseline: 1.2679x; 1.2679x over previous
"""CGConv GNN layer (CGCNNet + L1 sum head) on 8 Trainium2 NeuronCores.

Strategy (v3 — streaming, gather-free, software-pipelined):
  - Host sorts edges by destination node; each core owns a contiguous range of
    destination nodes (49 windows of 128 nodes), so segment-sums complete
    locally and no collectives are needed.
  - Host streams pre-transposed per-edge tiles: zA = [x_dst | x_src]^T
    (128 rows) and zB = [edge_attr | ones]^T (33 rows), sorted into
    (core, window, slot) order. No on-device gathers at all.
  - Per 128-edge tile: pre = zA^T @ W_A + zB^T @ W_B accumulated in PSUM
    (CGConv weights restacked so gate|core come out side by side, bias folded
    in via the ones row).
  - Nonlinearity with one resident ACT table (exp_and_others: tanh + exp):
      gate2 = 1 + tanh(pre_g / 2)            (= 2*sigmoid(pre_g))
      v     = exp(pre_c); sp = ln(1 + v)     (ln batched once per window)
      msg2  = gate2 * sp                     (one fused DVE op; = 2*msg)
  - Segment-sum via one-hot scatter matmuls (S built by DVE is_equal against
    an iota tile); the 2x is folded out in the epilogue:
      h = relu(0.5*agg2 + x_own), hacc += h.
  - Window stages are issued with a 1-window skew (stage A of window w+1
    ahead of the scatter of window w) so the tensor engine never waits on the
    ACT ln / DVE msg chain.
  - Final: pooled = ones^T @ hacc on PE; host sums the 8 per-core [64]
    vectors and applies the dense head.
"""

import os
import sys
import numpy as np

sys.path.insert(0, "/opt/trn_rl_repo")

import ml_dtypes

P = 128
NB = 8                  # tiles per PSUM batch
AGG_ROT = 4             # windows sharing the agg PSUM bank (rotating)

LAST_RESULTS = None     # test harness reads exec_time_ns from here


def _patch_tile_drain():
    """This walrus build rejects >1 semaphore wait on the tail-drain TPB_CTRL
    instruction. Split the waits across preceding NOPs."""
    import concourse.tile as tile_mod
    from concourse import mybir
    from concourse.vector_clock import ScopedClock

    if getattr(tile_mod.TileContext, "_drain_patched", False):
        return

    def _drain_and_barrier(self, tick_clock, wait_clock):
        nc = self.nc
        drain_inst = nc.sync.drain()
        wait_clock.add_sem_waits(
            drain_inst.ins, ScopedClock({None: tick_clock.global_clock})
        )
        si = drain_inst.ins.sync_info
        waits = list(si.on_wait or [])
        if len(waits) > 1:
            si.on_wait = waits[:1]
            extra = waits[1:]
            bb = nc.cur_bb.bb
            insts = bb.instructions
            carriers = []
            for w in extra:
                ni = nc.sync.nop(nofuse=True, hint="drain_wait_split")
                ni.ins.sync_info = mybir.SyncInfo(on_wait=[w], on_update=[])
                carriers.append(ni.ins)
            di = insts.index(drain_inst.ins)
            for c in carriers:
                insts.remove(c)
            insts[di:di] = carriers

        nc.all_engine_barrier()
        assert self.sems is not None
        popped = nc._tile_sem_poison_stack.pop()
        assert popped is self._sem_poison
        nc.clear_and_free_semaphores(list(self.sems.allocated().values()))
        nc.all_engine_barrier()

    tile_mod.TileContext._drain_and_barrier = _drain_and_barrier
    tile_mod.TileContext._drain_patched = True


def _chunks(n, size):
    out = []
    t = 0
    while t < n:
        out.append((t, min(t + size, n)))
        t += size
    return out


def _host_prep(inputs, ncores):
    bf16 = ml_dtypes.bfloat16
    x = np.asarray(inputs["x"], dtype=np.float32)
    ei = np.asarray(inputs["edge_index"], dtype=np.int64)
    ea = np.asarray(inputs["edge_attr"], dtype=np.float32)
    W_f = np.asarray(inputs["W_f"], dtype=np.float32)
    b_f = np.asarray(inputs["b_f"], dtype=np.float32)
    W_s = np.asarray(inputs["W_s"], dtype=np.float32)
    b_s = np.asarray(inputs["b_s"], dtype=np.float32)

    N, F = x.shape
    E = ei.shape[1]
    D = ea.shape[1]
    GC = 2 * F  # gate|core width

    nodes_pc = -(-((N + ncores - 1) // ncores) // P) * P   # per-core range
    wpc = nodes_pc // P
    nwin = ncores * wpc

    src, dst = ei[0], ei[1]
    core = dst // nodes_pc
    w_of = (dst % nodes_pc) // P
    d_rel = dst % P

    key = core * wpc + w_of
    order = np.argsort(key, kind="stable")
    key_s = key[order]
    src_s, dst_s, drel_s = src[order], dst[order], d_rel[order]

    counts = np.bincount(key_s, minlength=nwin).reshape(ncores, wpc)
    # per-window tile count: max over cores (SPMD: one program for all cores)
    nt_w = np.maximum(1, -(-counts.max(axis=0) // P))       # [wpc]
    off_w = np.zeros(wpc + 1, dtype=np.int64)
    np.cumsum(nt_w * P, out=off_w[1:])
    tot = int(off_w[-1])                                    # slots per core

    # slot index: window offset + position within window
    starts = np.zeros(nwin + 1, dtype=np.int64)
    np.cumsum(counts.reshape(-1), out=starts[1:])
    within = np.arange(E, dtype=np.int64) - starts[key_s]
    core_s = key_s // wpc
    w_s = key_s % wpc
    fidx = core_s * tot + off_w[w_s] + within

    x_bf = x.astype(bf16)
    ZAf = np.zeros((ncores * tot, P), dtype=bf16)
    ZAf[fidx, 0:F] = x_bf[dst_s]
    ZAf[fidx, F:2 * F] = x_bf[src_s]
    ZBf = np.zeros((ncores * tot, D + 1), dtype=bf16)
    ZBf[fidx, 0:D] = ea[order].astype(bf16)
    ZBf[:, D] = 1.0
    drelf = np.full(ncores * tot, -1.0, dtype=bf16)
    drelf[fidx] = drel_s.astype(bf16)

    # weights: rows = z dims, cols = [gate | core]
    W_A = np.concatenate([W_f[0:2 * F], W_s[0:2 * F]], axis=1).astype(bf16)
    W_B = np.concatenate(
        [np.concatenate([W_f[2 * F:], W_s[2 * F:]], axis=1),
         np.concatenate([b_f, b_s])[None, :]], axis=0).astype(bf16)

    iota_rep = np.tile(np.arange(P, dtype=np.float32), NB) \
        .astype(bf16)[None, :].repeat(P, axis=0)               # [P, NB*P]

    in_maps = []
    for c in range(ncores):
        lo, hi = c * tot, (c + 1) * tot
        zA = np.ascontiguousarray(ZAf[lo:hi].T)                # [128, tot]
        zB = np.ascontiguousarray(ZBf[lo:hi].T)                # [33, tot]
        drw = np.ascontiguousarray(
            drelf[lo:hi].reshape(tot // P, P).T)               # [128, tot//P]
        nlo = c * nodes_pc
        nhi = min(N, nlo + nodes_pc)
        xo = np.zeros((nodes_pc, F), dtype=bf16)
        if nhi > nlo:
            xo[: nhi - nlo] = x_bf[nlo:nhi]
        in_maps.append({
            "zA": zA,
            "zB": zB,
            "drw": drw,
            "x_own": np.ascontiguousarray(xo.reshape(wpc, P, F)),
            "W_A": W_A,
            "W_B": W_B,
            "iota": iota_rep,
        })

    cfg = dict(N=N, E=E, F=F, D=D, GC=GC, ncores=ncores, nodes_pc=nodes_pc,
               wpc=wpc, nt_w=[int(v) for v in nt_w],
               off_w=[int(v) for v in off_w], tot=tot)
    return in_maps, cfg


def _build_program(cfg):
    import concourse.bass as bass
    import concourse.tile as tile
    from concourse import bacc, mybir
    from contextlib import ExitStack

    _patch_tile_drain()

    F, GC, D = cfg["F"], cfg["GC"], cfg["D"]
    wpc, tot = cfg["wpc"], cfg["tot"]
    nt_w, off_w = cfg["nt_w"], cfg["off_w"]
    ncores = cfg["ncores"]
    ZB_R = D + 1
    f32, bf16 = mybir.dt.float32, mybir.dt.bfloat16
    AF = mybir.ActivationFunctionType
    ALU = mybir.AluOpType
    nt_max = max(nt_w)

    nc = bacc.Bacc("TRN2", target_bir_lowering=False, debug=False,
                   num_devices=ncores)

    t_in = {}
    t_in["zA"] = nc.dram_tensor("zA", [P, tot], bf16, kind="ExternalInput")
    t_in["zB"] = nc.dram_tensor("zB", [ZB_R, tot], bf16, kind="ExternalInput")
    t_in["drw"] = nc.dram_tensor("drw", [P, tot // P], bf16, kind="ExternalInput")
    t_in["x_own"] = nc.dram_tensor("x_own", [wpc, P, F], bf16, kind="ExternalInput")
    t_in["W_A"] = nc.dram_tensor("W_A", [P, GC], bf16, kind="ExternalInput")
    t_in["W_B"] = nc.dram_tensor("W_B", [ZB_R, GC], bf16, kind="ExternalInput")
    t_in["iota"] = nc.dram_tensor("iota", [P, NB * P], bf16, kind="ExternalInput")

    out_pooled = nc.dram_tensor("out_pooled", [1, F], f32, kind="ExternalOutput")

    with tile.TileContext(nc) as tc:
        with ExitStack() as ctx:
            cpool = ctx.enter_context(tc.tile_pool(name="consts", bufs=1))
            w_a_sb = cpool.tile([P, GC], bf16)
            nc.scalar.dma_start(w_a_sb[:], t_in["W_A"].ap()[:, :])
            w_b_sb = cpool.tile([ZB_R, GC], bf16)
            nc.scalar.dma_start(w_b_sb[:], t_in["W_B"].ap()[:, :])
            iota_sb = cpool.tile([P, NB, P], bf16)
            nc.scalar.dma_start(iota_sb[:], t_in["iota"].ap()[:, :])
            ones_sb = cpool.tile([P, 1], f32)
            nc.vector.memset(ones_sb[:], 1.0)
            hacc = cpool.tile([P, F], f32)
            nc.vector.memset(hacc[:], 0.0)

            zA_pool = ctx.enter_context(tc.tile_pool(name="zA", bufs=3))
            zB_pool = ctx.enter_context(tc.tile_pool(name="zB", bufs=3))
            dpool = ctx.enter_context(tc.tile_pool(name="drw", bufs=3))
            xpool = ctx.enter_context(tc.tile_pool(name="xw", bufs=3))
            tpool = ctx.enter_context(tc.tile_pool(name="tslab", bufs=3))
            vpool = ctx.enter_context(tc.tile_pool(name="vslab", bufs=3))
            sppool = ctx.enter_context(tc.tile_pool(name="spslab", bufs=3))
            mpool = ctx.enter_context(tc.tile_pool(name="msgslab", bufs=3))
            stpool = ctx.enter_context(tc.tile_pool(name="st", bufs=3))
            epool = ctx.enter_context(tc.tile_pool(name="epi", bufs=3))
            pre_ps = ctx.enter_context(
                tc.tile_pool(name="preps", bufs=3, space="PSUM"))
            agg_ps = ctx.enter_context(
                tc.tile_pool(name="aggps", bufs=1, space="PSUM"))
            pool_ps = ctx.enter_context(
                tc.tile_pool(name="poolps", bufs=1, space="PSUM"))
            aggslab = agg_ps.tile([P, AGG_ROT, F], f32, tag="aggslab")

            state = {}  # per in-flight window: tiles for stage C

            def stage_a(w):
                nt = nt_w[w]
                o0 = off_w[w]
                zA_t = zA_pool.tile([P, nt * P], bf16, tag="zA")
                nc.sync.dma_start(zA_t[:], t_in["zA"].ap()[:, o0:o0 + nt * P])
                zB_t = zB_pool.tile([ZB_R, nt * P], bf16, tag="zB")
                nc.sync.dma_start(zB_t[:], t_in["zB"].ap()[:, o0:o0 + nt * P])
                drw = dpool.tile([P, nt, 1], bf16, tag="drw")
                nc.sync.dma_start(
                    drw[:], t_in["drw"].ap()[:, o0 // P:o0 // P + nt])
                xw = xpool.tile([P, F], bf16, tag="xw")
                nc.sync.dma_start(xw[:], t_in["x_own"].ap()[w])

                tsl = tpool.tile([P, nt, F], bf16, tag="t")
                vsl = vpool.tile([P, nt, F], bf16, tag="v")
                for t0, t1 in _chunks(nt, NB):
                    nb = t1 - t0
                    pp = pre_ps.tile([P, NB, GC], f32, tag="pre")
                    for t in range(t0, t1):
                        i = t - t0
                        nc.tensor.matmul(pp[:, i, :],
                                         lhsT=zA_t[:, t * P:(t + 1) * P],
                                         rhs=w_a_sb[:], start=True, stop=False)
                        nc.tensor.matmul(pp[:, i, :],
                                         lhsT=zB_t[:, t * P:(t + 1) * P],
                                         rhs=w_b_sb[:], start=False, stop=True)
                    nc.scalar.activation(tsl[:, t0:t1, :], pp[:, :nb, 0:F],
                                         AF.Tanh, scale=0.5)
                    nc.scalar.activation(vsl[:, t0:t1, :], pp[:, :nb, F:GC],
                                         AF.Exp)
                state[w] = dict(drw=drw, xw=xw, tsl=tsl, vsl=vsl)

            def stage_b(w):
                s = state[w]
                nt = nt_w[w]
                spl = sppool.tile([P, nt, F], bf16, tag="sp")
                nc.scalar.activation(spl[:], s["vsl"][:], AF.Ln, bias=1.0)
                s["spl"] = spl

            def stage_c(w):
                s = state.pop(w)
                nt = nt_w[w]
                # one-hot tiles first (no cross-engine deps) so DVE never
                # blocks the scatter behind the next window's msg wait
                sts = []
                for t0, t1 in _chunks(nt, NB):
                    nb = t1 - t0
                    st = stpool.tile([P, NB, P], bf16, tag="st")
                    nc.vector.tensor_tensor(
                        st[:, :nb, :], iota_sb[:, :nb, :],
                        s["drw"][:, t0:t1, :].to_broadcast([P, nb, P]),
                        op=ALU.is_equal)
                    sts.append(st)
                msl = mpool.tile([P, nt, F], bf16, tag="msg")
                nc.vector.scalar_tensor_tensor(
                    msl[:], s["tsl"][:], 1.0, s["spl"][:],
                    op0=ALU.add, op1=ALU.mult)

                agg = aggslab[:, w % AGG_ROT, :]
                for bi, (t0, t1) in enumerate(_chunks(nt, NB)):
                    st = sts[bi]
                    for t in range(t0, t1):
                        nc.tensor.matmul(agg, lhsT=st[:, t - t0, :],
                                         rhs=msl[:, t, :],
                                         start=(t == 0), stop=(t == nt - 1))

                hsum = epool.tile([P, F], f32, tag="hsum")
                nc.vector.scalar_tensor_tensor(
                    hsum[:], agg, 0.5, s["xw"][:], op0=ALU.mult, op1=ALU.add)
                h = epool.tile([P, F], f32, tag="h")
                nc.vector.tensor_scalar_max(h[:], hsum[:], 0.0)
                nc.vector.tensor_tensor(hacc[:], hacc[:], h[:], op=ALU.add)

            for w in range(wpc + 1):
                if w < wpc:
                    stage_a(w)
                    stage_b(w)
                if w >= 1:
                    stage_c(w - 1)

            # pool over partitions
            pooled_ps = pool_ps.tile([1, F], f32, tag="pool")
            nc.tensor.matmul(pooled_ps[:], lhsT=ones_sb[:], rhs=hacc[:],
                             start=True, stop=True)
            pooled_sb = epool.tile([1, F], f32, tag="pooled")
            nc.vector.tensor_copy(pooled_sb[:], pooled_ps[:])
            nc.scalar.dma_start(out_pooled.ap()[:, :], pooled_sb[:])

    nc.compile()
    return nc


def kernel(**inputs):
    global LAST_RESULTS
    from concourse.bass_utils import run_bass_kernel_spmd

    ncores = 8
    in_maps, cfg = _host_prep(inputs, ncores)
    nc = _build_program(cfg)
    trace = bool(os.environ.get("BASS_TRACE"))
    res = run_bass_kernel_spmd(nc, in_maps, list(range(ncores)), trace=trace)
    LAST_RESULTS = res

    pooled = np.zeros(cfg["F"], dtype=np.float64)
    for c in range(ncores):
        pooled += res.results[c]["out_pooled"][0].astype(np.float64)
    W_dense = np.asarray(inputs["W_dense"], dtype=np.float64)
    b_dense = np.asarray(inputs["b_dense"], dtype=np.float64)
    out = pooled @ W_dense + b_dense
    return out.astype(np.float32)


# revision 13
# speedup vs baseline: 1.6830x; 1.3275x over previous
"""CGConv GNN layer (CGCNNet + L1 sum head) on 8 Trainium2 NeuronCores.

Strategy (v3 — streaming, gather-free, software-pipelined):
  - Host sorts edges by destination node; each core owns a contiguous range of
    destination nodes (49 windows of 128 nodes), so segment-sums complete
    locally and no collectives are needed.
  - Host streams pre-transposed per-edge tiles: zA = [x_dst | x_src]^T
    (128 rows) and zB = [edge_attr | ones]^T (33 rows), sorted into
    (core, window, slot) order. No on-device gathers at all.
  - Per 128-edge tile: pre = zA^T @ W_A + zB^T @ W_B accumulated in PSUM
    (CGConv weights restacked so gate|core come out side by side, bias folded
    in via the ones row).
  - Nonlinearity with one resident ACT table (exp_and_others: tanh + exp):
      gate2 = 1 + tanh(pre_g / 2)            (= 2*sigmoid(pre_g))
      v     = exp(pre_c); sp = ln(1 + v)     (ln batched once per window)
      msg2  = gate2 * sp                     (one fused DVE op; = 2*msg)
  - Segment-sum via one-hot scatter matmuls (S built by DVE is_equal against
    an iota tile); the 2x is folded out in the epilogue:
      h = relu(0.5*agg2 + x_own), hacc += h.
  - Window stages are issued with a 1-window skew (stage A of window w+1
    ahead of the scatter of window w) so the tensor engine never waits on the
    ACT ln / DVE msg chain.
  - Final: pooled = ones^T @ hacc on PE; host sums the 8 per-core [64]
    vectors and applies the dense head.
"""

import os
import sys
import numpy as np

sys.path.insert(0, "/opt/trn_rl_repo")

import ml_dtypes

P = 128
NB = 8                  # tiles per PSUM batch
AGG_ROT = 4             # windows sharing the agg PSUM bank (rotating)

LAST_RESULTS = None     # test harness reads exec_time_ns from here


def _patch_tile_drain():
    """This walrus build rejects >1 semaphore wait on the tail-drain TPB_CTRL
    instruction. Split the waits across preceding NOPs."""
    import concourse.tile as tile_mod
    from concourse import mybir
    from concourse.vector_clock import ScopedClock

    if getattr(tile_mod.TileContext, "_drain_patched", False):
        return

    def _drain_and_barrier(self, tick_clock, wait_clock):
        nc = self.nc
        drain_inst = nc.sync.drain()
        wait_clock.add_sem_waits(
            drain_inst.ins, ScopedClock({None: tick_clock.global_clock})
        )
        si = drain_inst.ins.sync_info
        waits = list(si.on_wait or [])
        if len(waits) > 1:
            si.on_wait = waits[:1]
            extra = waits[1:]
            bb = nc.cur_bb.bb
            insts = bb.instructions
            carriers = []
            for w in extra:
                ni = nc.sync.nop(nofuse=True, hint="drain_wait_split")
                ni.ins.sync_info = mybir.SyncInfo(on_wait=[w], on_update=[])
                carriers.append(ni.ins)
            di = insts.index(drain_inst.ins)
            for c in carriers:
                insts.remove(c)
            insts[di:di] = carriers

        nc.all_engine_barrier()
        assert self.sems is not None
        popped = nc._tile_sem_poison_stack.pop()
        assert popped is self._sem_poison
        nc.clear_and_free_semaphores(list(self.sems.allocated().values()))
        nc.all_engine_barrier()

    tile_mod.TileContext._drain_and_barrier = _drain_and_barrier
    tile_mod.TileContext._drain_patched = True


def _chunks(n, size):
    out = []
    t = 0
    while t < n:
        out.append((t, min(t + size, n)))
        t += size
    return out


def _host_prep(inputs, ncores):
    bf16 = ml_dtypes.bfloat16
    x = np.asarray(inputs["x"], dtype=np.float32)
    ei = np.asarray(inputs["edge_index"], dtype=np.int64)
    ea = np.asarray(inputs["edge_attr"], dtype=np.float32)
    W_f = np.asarray(inputs["W_f"], dtype=np.float32)
    b_f = np.asarray(inputs["b_f"], dtype=np.float32)
    W_s = np.asarray(inputs["W_s"], dtype=np.float32)
    b_s = np.asarray(inputs["b_s"], dtype=np.float32)

    N, F = x.shape
    E = ei.shape[1]
    D = ea.shape[1]
    GC = 2 * F  # gate|core width

    nodes_pc = -(-((N + ncores - 1) // ncores) // P) * P   # per-core range
    wpc = nodes_pc // P
    nwin = ncores * wpc

    src, dst = ei[0], ei[1]
    core = dst // nodes_pc
    w_of = (dst % nodes_pc) // P
    d_rel = dst % P

    key = core * wpc + w_of
    order = np.argsort(key, kind="stable")
    key_s = key[order]
    src_s, dst_s, drel_s = src[order], dst[order], d_rel[order]

    counts = np.bincount(key_s, minlength=nwin).reshape(ncores, wpc)
    # per-window tile count: max over cores (SPMD: one program for all cores)
    nt_w = np.maximum(1, -(-counts.max(axis=0) // P))       # [wpc]
    off_w = np.zeros(wpc + 1, dtype=np.int64)
    np.cumsum(nt_w * P, out=off_w[1:])
    tot = int(off_w[-1])                                    # slots per core

    # slot index: window offset + position within window
    starts = np.zeros(nwin + 1, dtype=np.int64)
    np.cumsum(counts.reshape(-1), out=starts[1:])
    within = np.arange(E, dtype=np.int64) - starts[key_s]
    core_s = key_s // wpc
    w_s = key_s % wpc
    fidx = core_s * tot + off_w[w_s] + within

    x_bf = x.astype(bf16)
    ZAf = np.zeros((ncores * tot, P), dtype=bf16)
    ZAf[fidx, 0:F] = x_bf[dst_s]
    ZAf[fidx, F:2 * F] = x_bf[src_s]
    ZBf = np.zeros((ncores * tot, D + 1), dtype=bf16)
    ZBf[fidx, 0:D] = ea[order].astype(bf16)
    ZBf[:, D] = 1.0
    drelf = np.full(ncores * tot, -1.0, dtype=bf16)
    drelf[fidx] = drel_s.astype(bf16)

    # weights: rows = z dims, cols = [gate | core]
    W_A = np.concatenate([W_f[0:2 * F], W_s[0:2 * F]], axis=1).astype(bf16)
    W_B = np.concatenate(
        [np.concatenate([W_f[2 * F:], W_s[2 * F:]], axis=1),
         np.concatenate([b_f, b_s])[None, :]], axis=0).astype(bf16)

    iota_rep = np.tile(np.arange(P, dtype=np.float32), NB) \
        .astype(bf16)[None, :].repeat(P, axis=0)               # [P, NB*P]

    in_maps = []
    for c in range(ncores):
        lo, hi = c * tot, (c + 1) * tot
        zA = np.ascontiguousarray(ZAf[lo:hi].T)                # [128, tot]
        zB = np.ascontiguousarray(ZBf[lo:hi].T)                # [33, tot]
        drw = np.ascontiguousarray(
            drelf[lo:hi].reshape(tot // P, P).T)               # [128, tot//P]
        nlo = c * nodes_pc
        nhi = min(N, nlo + nodes_pc)
        xo = np.zeros((nodes_pc, F), dtype=bf16)
        if nhi > nlo:
            xo[: nhi - nlo] = x_bf[nlo:nhi]
        in_maps.append({
            "zA": zA,
            "zB": zB,
            "drw": drw,
            "x_own": np.ascontiguousarray(xo.reshape(wpc, P, F)),
            "W_A": W_A,
            "W_B": W_B,
            "iota": iota_rep,
        })

    cfg = dict(N=N, E=E, F=F, D=D, GC=GC, ncores=ncores, nodes_pc=nodes_pc,
               wpc=wpc, nt_w=[int(v) for v in nt_w],
               off_w=[int(v) for v in off_w], tot=tot)
    return in_maps, cfg


def _build_program(cfg):
    import concourse.bass as bass
    import concourse.tile as tile
    from concourse import bacc, mybir
    from contextlib import ExitStack

    _patch_tile_drain()

    F, GC, D = cfg["F"], cfg["GC"], cfg["D"]
    wpc, tot = cfg["wpc"], cfg["tot"]
    nt_w, off_w = cfg["nt_w"], cfg["off_w"]
    ncores = cfg["ncores"]
    ZB_R = D + 1
    f32, bf16 = mybir.dt.float32, mybir.dt.bfloat16
    AF = mybir.ActivationFunctionType
    ALU = mybir.AluOpType
    nt_max = max(nt_w)

    nc = bacc.Bacc("TRN2", target_bir_lowering=False, debug=False,
                   num_devices=ncores)

    t_in = {}
    t_in["zA"] = nc.dram_tensor("zA", [P, tot], bf16, kind="ExternalInput")
    t_in["zB"] = nc.dram_tensor("zB", [ZB_R, tot], bf16, kind="ExternalInput")
    t_in["drw"] = nc.dram_tensor("drw", [P, tot // P], bf16, kind="ExternalInput")
    t_in["x_own"] = nc.dram_tensor("x_own", [wpc, P, F], bf16, kind="ExternalInput")
    t_in["W_A"] = nc.dram_tensor("W_A", [P, GC], bf16, kind="ExternalInput")
    t_in["W_B"] = nc.dram_tensor("W_B", [ZB_R, GC], bf16, kind="ExternalInput")
    t_in["iota"] = nc.dram_tensor("iota", [P, NB * P], bf16, kind="ExternalInput")

    out_pooled = nc.dram_tensor("out_pooled", [1, F], f32, kind="ExternalOutput")

    with tile.TileContext(nc) as tc:
        with ExitStack() as ctx:
            cpool = ctx.enter_context(tc.tile_pool(name="consts", bufs=1))
            w_a_sb = cpool.tile([P, GC], bf16)
            nc.scalar.dma_start(w_a_sb[:], t_in["W_A"].ap()[:, :])
            w_b_sb = cpool.tile([ZB_R, GC], bf16)
            nc.scalar.dma_start(w_b_sb[:], t_in["W_B"].ap()[:, :])
            iota_sb = cpool.tile([P, NB, P], bf16)
            nc.scalar.dma_start(iota_sb[:], t_in["iota"].ap()[:, :])
            ones_sb = cpool.tile([P, 1], f32)
            nc.vector.memset(ones_sb[:], 1.0)
            hacc = cpool.tile([P, F], f32)
            nc.vector.memset(hacc[:], 0.0)

            zA_pool = ctx.enter_context(tc.tile_pool(name="zA", bufs=3))
            zB_pool = ctx.enter_context(tc.tile_pool(name="zB", bufs=3))
            dpool = ctx.enter_context(tc.tile_pool(name="drw", bufs=4))
            xpool = ctx.enter_context(tc.tile_pool(name="xw", bufs=4))
            tpool = ctx.enter_context(tc.tile_pool(name="tslab", bufs=4))
            vpool = ctx.enter_context(tc.tile_pool(name="vslab", bufs=3))
            sppool = ctx.enter_context(tc.tile_pool(name="spslab", bufs=4))
            mpool = ctx.enter_context(tc.tile_pool(name="msgslab", bufs=4))
            stpool = ctx.enter_context(tc.tile_pool(name="st", bufs=3))
            epool = ctx.enter_context(tc.tile_pool(name="epi", bufs=3))
            pre_ps = ctx.enter_context(
                tc.tile_pool(name="preps", bufs=3, space="PSUM"))
            agg_ps = ctx.enter_context(
                tc.tile_pool(name="aggps", bufs=1, space="PSUM"))
            pool_ps = ctx.enter_context(
                tc.tile_pool(name="poolps", bufs=1, space="PSUM"))
            aggslab = agg_ps.tile([P, AGG_ROT, F], f32, tag="aggslab")

            state = {}  # per in-flight window: tiles for stage C

            def stage_a(w):
                nt = nt_w[w]
                o0 = off_w[w]
                zA_t = zA_pool.tile([P, nt * P], bf16, tag="zA")
                nc.sync.dma_start(zA_t[:], t_in["zA"].ap()[:, o0:o0 + nt * P])
                zB_t = zB_pool.tile([ZB_R, nt * P], bf16, tag="zB")
                nc.sync.dma_start(zB_t[:], t_in["zB"].ap()[:, o0:o0 + nt * P])
                drw = dpool.tile([P, nt, 1], bf16, tag="drw")
                nc.sync.dma_start(
                    drw[:], t_in["drw"].ap()[:, o0 // P:o0 // P + nt])
                xw = xpool.tile([P, F], bf16, tag="xw")
                nc.sync.dma_start(xw[:], t_in["x_own"].ap()[w])

                tsl = tpool.tile([P, nt, F], bf16, tag="t")
                vsl = vpool.tile([P, nt, F], bf16, tag="v")
                for t0, t1 in _chunks(nt, NB):
                    nb = t1 - t0
                    pp = pre_ps.tile([P, NB, GC], f32, tag="pre")
                    # group by contraction dim: switching K (128<->33) costs
                    # ~180ns on PE, so do all zA then all zB. start=True marks
                    # the whole 2KB PSUM bank (4 regions) pending-zero, so it
                    # goes only on the first matmul touching each bank; the
                    # other zA writes then overwrite-fresh and the zB pass
                    # accumulates.
                    bank_first = {0, 4}
                    bank_last = {min(nb, 4) - 1, nb - 1}
                    for t in range(t0, t1):
                        i = t - t0
                        nc.tensor.matmul(pp[:, i, :],
                                         lhsT=zA_t[:, t * P:(t + 1) * P],
                                         rhs=w_a_sb[:],
                                         start=(i in bank_first), stop=False)
                    for t in range(t0, t1):
                        i = t - t0
                        nc.tensor.matmul(pp[:, i, :],
                                         lhsT=zB_t[:, t * P:(t + 1) * P],
                                         rhs=w_b_sb[:],
                                         start=False, stop=(i in bank_last))
                    nc.scalar.activation(tsl[:, t0:t1, :], pp[:, :nb, 0:F],
                                         AF.Tanh, scale=0.5)
                    nc.scalar.activation(vsl[:, t0:t1, :], pp[:, :nb, F:GC],
                                         AF.Exp)
                state[w] = dict(drw=drw, xw=xw, tsl=tsl, vsl=vsl)

            def stage_b(w):
                s = state[w]
                nt = nt_w[w]
                spl = sppool.tile([P, nt, F], bf16, tag="sp")
                nc.scalar.activation(spl[:], s["vsl"][:], AF.Ln, bias=1.0)
                s["spl"] = spl

            def stage_c(w):
                s = state.pop(w)
                nt = nt_w[w]
                # one-hot tiles first (no cross-engine deps) so DVE never
                # blocks the scatter behind the next window's msg wait
                sts = []
                for t0, t1 in _chunks(nt, NB):
                    nb = t1 - t0
                    st = stpool.tile([P, NB, P], bf16, tag="st")
                    nc.vector.tensor_tensor(
                        st[:, :nb, :], iota_sb[:, :nb, :],
                        s["drw"][:, t0:t1, :].to_broadcast([P, nb, P]),
                        op=ALU.is_equal)
                    sts.append(st)
                msl = mpool.tile([P, nt, F], bf16, tag="msg")
                nc.vector.scalar_tensor_tensor(
                    msl[:], s["tsl"][:], 1.0, s["spl"][:],
                    op0=ALU.add, op1=ALU.mult)

                agg = aggslab[:, w % AGG_ROT, :]
                for bi, (t0, t1) in enumerate(_chunks(nt, NB)):
                    st = sts[bi]
                    for t in range(t0, t1):
                        nc.tensor.matmul(agg, lhsT=st[:, t - t0, :],
                                         rhs=msl[:, t, :],
                                         start=(t == 0), stop=(t == nt - 1))

                hsum = epool.tile([P, F], f32, tag="hsum")
                nc.vector.scalar_tensor_tensor(
                    hsum[:], agg, 0.5, s["xw"][:], op0=ALU.mult, op1=ALU.add)
                h = epool.tile([P, F], f32, tag="h")
                nc.vector.tensor_scalar_max(h[:], hsum[:], 0.0)
                nc.vector.tensor_tensor(hacc[:], hacc[:], h[:], op=ALU.add)

            # paired-window schedule: A(2k) C(2k-2) A(2k+1) C(2k-1) B(2k) B(2k+1)
            # -> the two ln table switches amortize over two windows, and the
            # scatter of window w runs two A-stages after its own A (msg ready)
            npairs = (wpc + 1) // 2
            for k in range(npairs + 1):
                w0, w1 = 2 * k, 2 * k + 1
                if w0 < wpc:
                    stage_a(w0)
                if k >= 1:
                    stage_c(2 * k - 2)
                if w1 < wpc:
                    stage_a(w1)
                if k >= 1 and 2 * k - 1 < wpc:
                    stage_c(2 * k - 1)
                if w0 < wpc:
                    stage_b(w0)
                if w1 < wpc:
                    stage_b(w1)

            # pool over partitions
            pooled_ps = pool_ps.tile([1, F], f32, tag="pool")
            nc.tensor.matmul(pooled_ps[:], lhsT=ones_sb[:], rhs=hacc[:],
                             start=True, stop=True)
            pooled_sb = epool.tile([1, F], f32, tag="pooled")
            nc.vector.tensor_copy(pooled_sb[:], pooled_ps[:])
            nc.scalar.dma_start(out_pooled.ap()[:, :], pooled_sb[:])

    nc.compile()
    return nc


def kernel(**inputs):
    global LAST_RESULTS
    from concourse.bass_utils import run_bass_kernel_spmd

    ncores = 8
    in_maps, cfg = _host_prep(inputs, ncores)
    nc = _build_program(cfg)
    trace = bool(os.environ.get("BASS_TRACE"))
    res = run_bass_kernel_spmd(nc, in_maps, list(range(ncores)), trace=trace)
    LAST_RESULTS = res

    pooled = np.zeros(cfg["F"], dtype=np.float64)
    for c in range(ncores):
        pooled += res.results[c]["out_pooled"][0].astype(np.float64)
    W_dense = np.asarray(inputs["W_dense"], dtype=np.float64)
    b_dense = np.asarray(inputs["b_dense"], dtype=np.float64)
    out = pooled @ W_dense + b_dense
    return out.astype(np.float32)
